# revision 37
# baseline (speedup 1.0000x reference)
"""Trainium2 Bass kernel for nn_Attention_11063835754934.

reference:
    qp  = q @ Wq.T                      [B, NQ, D]
    S   = qp @ k.T / sqrt(D) + log(mask)
    out = softmax(S) @ v

Identities used:
  - q @ Wq.T @ k.T == q @ (k @ Wq).T  -> project K instead of Q.
  - 1/sqrt(D) is folded into Wq at setup, so the exp activation runs with
    scale=1.
  - exp(S)*mask == exp(S + log mask): the log(mask) add becomes a multiply
    after the exponent.  Softmax max-subtraction is skipped (S ~ N(0,1)).

Sharding: data-parallel on batch: B=16 over 8 cores -> 2 batches per core.

Layout trick ("(p t)"): q, k and v are loaded with partition p holding rows
16p..16p+15 (8KB contiguous per partition -> fat DMA descriptors, done as
SWDGE fp32->bf16 in-flight casts).  Thin-packet (512B-descriptor) loads
lose the SDMA packet round-robin ~8:1 against the mask stream and crawl at
~45GB/s; fat packets don't.  PE-transposing free-block t of that staging
tile yields columns in (t,p)-permuted order; the PSUM->SBUF assembly copy
un-permutes via a strided AP (transpose_blocks_ptfix), so qT/kT/vT and all
downstream tiles stay in NATURAL row order.  That keeps mask reads dense
(2MB contiguous HBM blocks at ~line rate; a permuted mask layout reads a
16KB-every-128KB comb at only ~76% efficiency) and keeps the S-matmul rhs
contiguous (a strided PE moving operand runs ~3.2x slower).

Per (batch, q-tile), software-pipelined, two nk-halves of 1024:
  S[128,1024]   = qT_qt.T @ kpT          (2 matmuls N=512, bf16)
  P             = exp(S)                 (ScalarE, PSUM->SBUF, bf16 out;
                                          1/sqrt(D) folded into Wq)
  Pm            = P * mask_tile          (VectorE 2x bf16)
  PmT           = transpose(Pm)          (16 PE transposes -> bf16 PSUM,
                                          2 [128,1024] copies to SBUF)
  o[128,129]    = sum_j PmT_j.T @ [v_j|1] (16 accumulating matmuls; col 128
                                          = softmax denominator)
  out           = o[:, :128] * 1/o[:,128] (VectorE reciprocal + ScalarE mul)
Output rows are written into an SG-tile group and stored every SG q-tiles.
The next batch's loads are issued at qt=T/2-2 and its setup (transposes +
k-projection) is emitted at qt=T/2+2 so the Tile scheduler interleaves it
into the current batch's pipeline instead of stalling PE at the boundary.
"""
import os
import sys

for _p in ("/opt/trn_rl_repo", "/root/.axon_site/_ro/trn_rl_repo"):
    if os.path.isdir(_p) and _p not in sys.path:
        sys.path.append(_p)

import numpy as np

import concourse.bass as bass
import concourse.tile as tile
from concourse import mybir
from concourse.masks import make_identity

B, N, D = 16, 2048, 128
NCORES = 8
NB = B // NCORES          # batches per core
T = N // 128              # 16 tiles of 128 rows
SCALE = float(1.0 / np.sqrt(D))
BF16 = mybir.dt.bfloat16
F32 = mybir.dt.float32

NACT = int(os.environ.get("KERNEL_NACT", "0"))      # PmT copy groups on ACT
LAG = int(os.environ.get("KERNEL_LAG", "1"))        # stage2 lag in q-tiles
MGROUP = int(os.environ.get("KERNEL_MGROUP", "2"))  # q-tiles per mask DMA
MASKB = int(os.environ.get("KERNEL_MASKB", "4"))
WORKB = int(os.environ.get("KERNEL_WORKB", "2"))
PREF = int(os.environ.get("KERNEL_PREF", "2"))      # mask groups issued ahead
POOLMUL = int(os.environ.get("KERNEL_POOLMUL", "0"))  # every n-th mul on Pool
SG = int(os.environ.get("KERNEL_SG", "4"))          # q-tiles per out store
PSB = int(os.environ.get("KERNEL_PSB", "2"))        # ps_pool bufs
KPT = os.environ.get("KERNEL_KPT", "pt2")           # "pt2" | "pt" | "nat"
QPT = os.environ.get("KERNEL_QPT", "1") == "1"      # q "(p t)" layout
FAKEMASK = os.environ.get("KERNEL_FAKEMASK", "0") == "1"

MAXW = 1  # container walrus rejects >1 sync-wait per instruction


def _split_sync_waits(nc, maxw=MAXW):
    for bb in nc.main_func.blocks:
        out = []
        for ins in bb.instructions:
            si = ins.sync_info
            if si is not None and si.on_wait and len(si.on_wait) > maxw:
                waits = list(si.on_wait)
                extra, keep = waits[:-maxw], waits[-maxw:]
                while extra:
                    chunk, extra = extra[:maxw], extra[maxw:]
                    out.append(mybir.InstNoOp(
                        name=f"I-splitw-{nc.next_id()}",
                        engine=ins.engine, ins=[], outs=[],
                        text_hint="split_sync_waits", bass_nofuse=True,
                        sync_info=mybir.SyncInfo(on_wait=chunk, on_update=[]),
                    ))
                si.on_wait = keep
            out.append(ins)
        bb.instructions = out


def build_nc(reps: int = 1, split_waits: bool = True):
    nc = bass.Bass("TRN2", target_bir_lowering=False, debug=False,
                   num_devices=NCORES)
    qd = nc.dram_tensor("q", [NB, N, D], F32, kind="ExternalInput").ap()
    kd = nc.dram_tensor("k", [NB, N, D], F32, kind="ExternalInput").ap()
    vd = nc.dram_tensor("v", [NB, N, D], F32, kind="ExternalInput").ap()
    md = nc.dram_tensor("mask", [NB, N, N], F32, kind="ExternalInput").ap()
    wqd = nc.dram_tensor("Wq", [D, D], F32, kind="ExternalInput").ap()
    od = nc.dram_tensor("out", [NB, N, D], F32, kind="ExternalOutput").ap()

    NG = T // MGROUP  # mask groups per batch

    with tile.TileContext(nc) as tc:
        with (
            tc.tile_pool(name="const", bufs=1) as const,
            tc.tile_pool(name="stage", bufs=2) as stage,
            tc.tile_pool(name="perb", bufs=2) as perb,
            tc.tile_pool(name="maskp", bufs=MASKB) as maskp,
            tc.tile_pool(name="work", bufs=WORKB) as work,
            tc.tile_pool(name="outp", bufs=3) as outp,
            tc.tile_pool(name="ps_pool", bufs=PSB, space="PSUM") as ps_pool,
            tc.tile_pool(name="pt_pool", bufs=2, space="PSUM") as pt_pool,
            tc.tile_pool(name="po_pool", bufs=2, space="PSUM") as po_pool,
        ):
            ident = const.tile([128, 128], BF16, tag="ident")
            make_identity(nc, ident)
            wq_raw = const.tile([128, 128], BF16, tag="wq_raw")
            nc.gpsimd.dma_start(out=wq_raw, in_=wqd)  # fp32 -> bf16 cast
            wq_sc = const.tile([128, 128], BF16, tag="wq_sc")
            nc.vector.tensor_scalar_mul(wq_sc, wq_raw, SCALE)

            def transpose_blocks(dst3, src3, parity):
                # src3: [128, T, 128] staging; dst3: [128, T, 128] with
                # dst3[:, t, :] = src3[:, t, :].T
                G = 8
                for tg in range(T // G):
                    pt = pt_pool.tile([128, G * 128], BF16, tag="pt")
                    for j in range(G):
                        nc.tensor.transpose(
                            pt[:, j * 128:(j + 1) * 128],
                            src3[:, tg * G + j, :], ident)
                    dslice = dst3[:, tg * G:(tg + 1) * G, :]
                    if (tg + parity) % 2 == 0:
                        nc.scalar.copy(out=dslice, in_=pt)
                    else:
                        nc.vector.tensor_copy(out=dslice, in_=pt)

            def issue_loads(b):
                """Issue the q/k/v load DMAs for batch b (q via SWDGE bf16
                cast in the "(p t)" fat-descriptor layout; k/v via HWDGE
                fp32).  Called one batch ahead so the loads don't queue
                behind batch b's mask stream on the SWDGE queue."""
                if QPT:
                    qsrc = qd[b].rearrange("(p t) d -> p t d", t=T)
                else:
                    qsrc = qd[b].rearrange("(t p) d -> p t d", p=128)
                qstage = stage.tile([128, T, 128], BF16, tag="qstage")
                nc.gpsimd.dma_start(out=qstage, in_=qsrc)
                if KPT in ("pt", "pt2"):
                    ksrc = kd[b].rearrange("(p t) d -> p t d", t=T)
                    kstage = stage.tile([128, T, 128], BF16, tag="kstage")
                    nc.gpsimd.dma_start(out=kstage, in_=ksrc)
                    kstage_f = None
                else:
                    # HWDGE fp32 load + DVE cast (SWDGE 512B-desc gen too slow
                    # for the natural layout; strided PE rhs too slow for pt)
                    kstage_f = stage.tile([128, T, 128], F32, tag="kstage_f")
                    nc.sync.dma_start(
                        out=kstage_f,
                        in_=kd[b].rearrange("(t p) d -> p t d", p=128))
                    kstage = None
                # v also goes through the SWDGE fat-descriptor path: a 512B-
                # packet HWDGE load loses the SDMA round-robin ~8:1 against
                # the mask stream's fat packets and its tail crawls for ~25us,
                # stalling whichever engine queue holds the v1 build.  The
                # natural [m, d] tiles are recovered with two PE-transpose
                # rounds in emit_setup.
                vpt = stage.tile([128, T, 128], BF16, tag="vpt")
                nc.gpsimd.dma_start(
                    out=vpt, in_=vd[b].rearrange("(p t) d -> p t d", t=T))
                return qstage, kstage, kstage_f, vpt

            def transpose_blocks_ptfix(dst2, src3, parity=0):
                """Transpose "(p t)"-staged src into NATURAL column order:
                transpose block t yields columns {16p+t}; the PSUM->SBUF copy
                scatters them via a strided AP (inner stride 16 elems)."""
                G = 8
                dstv = dst2.rearrange("d (p s) -> d p s", p=128)
                for tg in range(T // G):
                    pt = pt_pool.tile([128, G * 128], BF16, tag="pt")
                    for j in range(G):
                        nc.tensor.transpose(
                            pt[:, j * 128:(j + 1) * 128],
                            src3[:, tg * G + j, :], ident)
                    src = pt.rearrange("d (j p) -> d p j", p=128)
                    dst = dstv[:, :, tg * G:(tg + 1) * G]
                    if (tg + parity) % 2 == 0:
                        nc.scalar.copy(out=dst, in_=src)
                    else:
                        nc.vector.tensor_copy(out=dst, in_=src)

            def emit_setup(b, loads):
                """Transpose q/k, project k, build v1 (+ ones column)."""
                qstage, kstage, kstage_f, vpt = loads
                if kstage is None:
                    kstage = stage.tile([128, T, 128], BF16, tag="kstage")
                    nc.vector.tensor_copy(out=kstage, in_=kstage_f)

                qT = perb.tile([128, N], BF16, tag="qT")
                if QPT:
                    transpose_blocks_ptfix(qT, qstage, parity=0)
                else:
                    transpose_blocks(
                        qT.rearrange("d (t p) -> d t p", p=128), qstage, 0)
                kT = stage.tile([128, N], BF16, tag="kT")
                if KPT == "pt2":
                    transpose_blocks_ptfix(kT, kstage, parity=1)
                else:
                    transpose_blocks(
                        kT.rearrange("d (t p) -> d t p", p=128), kstage, 1)

                # kpT[e, c] = sum_d (Wq[d, e]/sqrt(D)) * kT[d, c]
                kpT = perb.tile([128, N], BF16, tag="kpT")
                for c in range(N // 512):
                    pm = pt_pool.tile([128, 512], F32, tag="pt")
                    nc.tensor.matmul(pm, lhsT=wq_sc,
                                     rhs=kT[:, c * 512:(c + 1) * 512],
                                     start=True, stop=True)
                    if c % 2 == 0:
                        nc.scalar.copy(out=kpT[:, c * 512:(c + 1) * 512],
                                       in_=pm)
                    else:
                        nc.vector.tensor_copy(
                            out=kpT[:, c * 512:(c + 1) * 512], in_=pm)

                # v: "(p t)" staging -> natural vT -> per-j [m, d] tiles via a
                # second transpose round (ones col appended for the softmax
                # denominator).
                vT = stage.tile([128, N], BF16, tag="vT")
                transpose_blocks_ptfix(vT, vpt, parity=1)
                v1 = perb.tile([128, T, 129], BF16, tag="v1")
                G = 8
                for tg in range(T // G):
                    pt = pt_pool.tile([128, G * 128], BF16, tag="pt")
                    for j in range(G):
                        t_idx = tg * G + j
                        nc.tensor.transpose(
                            pt[:, j * 128:(j + 1) * 128],
                            vT[:, t_idx * 128:(t_idx + 1) * 128], ident)
                    dst = v1[:, tg * G:(tg + 1) * G, 0:128]
                    src = pt.rearrange("d (j p) -> d j p", p=128)
                    if tg % 2 == 0:
                        nc.vector.tensor_copy(out=dst, in_=src)
                    else:
                        nc.scalar.copy(out=dst, in_=src)
                nc.vector.memset(v1[:, :, 128:129], 1.0)
                return qT, kpT, v1

            for _rep in range(reps):
                mask_tiles = {}

                def issue_mask(gi):
                    # mask rows stay NATURAL (dense 2MB HBM blocks): the "(p
                    # s)" permuted layout reads a 16KB-every-128KB comb that
                    # drops HBM efficiency to ~76%.
                    b, g = gi // NG, gi % NG
                    msk = maskp.tile([128, MGROUP, N], BF16, tag="msk")
                    msrc = md[b].rearrange("(s p) c -> p s c", p=128)
                    nc.gpsimd.dma_start(
                        out=msk,
                        in_=msrc[:, g * MGROUP:(g + 1) * MGROUP, :])
                    mask_tiles[gi] = msk

                def emit_stage1(b, qt, qT, kpT, v1):
                    gi = b * NG + qt // MGROUP
                    if qt % MGROUP == 0:
                        ahead = 1 if gi == 0 else PREF
                        want = 0 if FAKEMASK else min(gi + ahead, NB * NG - 1)
                        for i in range(max(mask_tiles) + 1 if mask_tiles
                                       else 0, want + 1):
                            issue_mask(i)
                    msk = mask_tiles[0 if FAKEMASK else gi]

                    P = work.tile([128, N], BF16, tag="P")
                    Pm = work.tile([128, N], BF16, tag="Pm")
                    for h in range(2):
                        hsl = slice(h * 1024, (h + 1) * 1024)
                        ps = ps_pool.tile([128, 1024], F32, tag="ps")
                        for cc in range(2):
                            c = 2 * h + cc
                            rhs = kpT[:, c * 512:(c + 1) * 512]
                            nc.tensor.matmul(
                                ps[:, cc * 512:(cc + 1) * 512],
                                lhsT=qT[:, qt * 128:(qt + 1) * 128], rhs=rhs,
                                start=True, stop=True)
                        nc.scalar.activation(P[:, hsl], ps,
                                             mybir.ActivationFunctionType.Exp)
                        mi = qt * 2 + h
                        eng = (nc.gpsimd if POOLMUL and mi % POOLMUL ==
                               POOLMUL - 1 else nc.vector)
                        eng.tensor_mul(Pm[:, hsl], P[:, hsl],
                                       msk[:, qt % MGROUP, hsl])
                    return Pm

                def emit_stage2(b, qt, Pm, v1, ogroup):
                    PmT = work.tile([128, T, 128], BF16, tag="PmT")
                    G = 8
                    for tg in range(T // G):
                        pt = pt_pool.tile([128, G * 128], BF16, tag="pt")
                        for j in range(G):
                            t_idx = tg * G + j
                            nc.tensor.transpose(
                                pt[:, j * 128:(j + 1) * 128],
                                Pm[:, t_idx * 128:(t_idx + 1) * 128], ident)
                        dslice = PmT[:, tg * G:(tg + 1) * G, :]
                        if (tg + qt) % 4 < NACT:
                            nc.scalar.copy(out=dslice, in_=pt)
                        else:
                            nc.vector.tensor_copy(out=dslice, in_=pt)

                    po = po_pool.tile([128, 129], F32, tag="po")
                    for j in range(T):
                        nc.tensor.matmul(po, lhsT=PmT[:, j, :],
                                         rhs=v1[:, j, :],
                                         start=(j == 0), stop=(j == T - 1))

                    rinv = outp.tile([128, 1], F32, tag="rinv")
                    nc.vector.reciprocal(rinv, po[:, 128:129])
                    if qt % SG == 0:
                        ogroup[0] = outp.tile([128, SG, 128], F32,
                                              tag="osb", name="osb")
                    nc.scalar.mul(ogroup[0][:, qt % SG, :], po[:, 0:128], rinv)
                    if qt % SG == SG - 1:
                        qt0 = qt - SG + 1
                        odst = od[b].rearrange("(s p) d -> p s d", p=128)
                        nc.sync.dma_start(
                            out=odst[:, qt0:qt0 + SG, :], in_=ogroup[0])

                from collections import deque
                pending = deque()
                ogroup = [None]
                if FAKEMASK:
                    issue_mask(0)
                loads = issue_loads(0)
                built = emit_setup(0, loads)
                for b in range(NB):
                    qT, kpT, v1 = built
                    for qt in range(T):
                        Pm = emit_stage1(b, qt, qT, kpT, v1)
                        if qt == T // 2 - 2 and b + 1 < NB:
                            loads = issue_loads(b + 1)
                        # emit the next batch's setup build mid-batch so its
                        # PE transposes / kproj interleave into this batch's
                        # pipeline instead of stalling PE at the boundary
                        if qt == T // 2 + 2 and b + 1 < NB:
                            built = emit_setup(b + 1, loads)
                        pending.append((b, qt, Pm, v1))
                        if len(pending) > LAG:
                            emit_stage2(*pending.popleft(), ogroup)
                while pending:
                    emit_stage2(*pending.popleft(), ogroup)

    if split_waits:
        _split_sync_waits(nc)
    return nc


_CACHE = {}


def _get_nc(reps=1):
    if reps not in _CACHE:
        _CACHE[reps] = build_nc(reps)
    return _CACHE[reps]


def kernel(q, k, v, mask, Wq):
    from concourse.bass_utils import run_bass_kernel_spmd
    nc = _get_nc()
    in_maps = []
    for c in range(NCORES):
        sl = slice(c * NB, (c + 1) * NB)
        in_maps.append({
            "q": np.ascontiguousarray(q[sl]),
            "k": np.ascontiguousarray(k[sl]),
            "v": np.ascontiguousarray(v[sl]),
            "mask": np.ascontiguousarray(mask[sl]),
            "Wq": np.ascontiguousarray(Wq),
        })
    res = run_bass_kernel_spmd(nc, in_maps, list(range(NCORES)))
    out = np.concatenate([res.results[c]["out"] for c in range(NCORES)], axis=0)
    return out.astype(np.float32)


# revision 52
# speedup vs baseline: 1.0039x; 1.0039x over previous
"""Trainium2 Bass kernel for nn_Attention_11063835754934.

reference:
    qp  = q @ Wq.T                      [B, NQ, D]
    S   = qp @ k.T / sqrt(D) + log(mask)
    out = softmax(S) @ v

Identities used:
  - q @ Wq.T @ k.T == q @ (k @ Wq).T  -> project K instead of Q.
  - 1/sqrt(D) is folded into Wq at setup, so the exp activation runs with
    scale=1.
  - exp(S)*mask == exp(S + log mask): the log(mask) add becomes a multiply
    after the exponent.  Softmax max-subtraction is skipped (S ~ N(0,1)).

Sharding: data-parallel on batch: B=16 over 8 cores -> 2 batches per core.

Layout trick ("(p t)"): q, k and v are loaded with partition p holding rows
16p..16p+15 (8KB contiguous per partition -> fat DMA descriptors, done as
SWDGE fp32->bf16 in-flight casts).  Thin-packet (512B-descriptor) loads
lose the SDMA packet round-robin ~8:1 against the mask stream and crawl at
~45GB/s; fat packets don't.  PE-transposing free-block t of that staging
tile yields columns in (t,p)-permuted order; the PSUM->SBUF assembly copy
un-permutes via a strided AP (transpose_blocks_ptfix), so qT/kT/vT and all
downstream tiles stay in NATURAL row order.  That keeps mask reads dense
(2MB contiguous HBM blocks at ~line rate; a permuted mask layout reads a
16KB-every-128KB comb at only ~76% efficiency) and keeps the S-matmul rhs
contiguous (a strided PE moving operand runs ~3.2x slower).

Per (batch, q-tile), software-pipelined, two nk-halves of 1024:
  S[128,1024]   = qT_qt.T @ kpT          (2 matmuls N=512, bf16)
  P             = exp(S)                 (ScalarE, PSUM->SBUF, bf16 out;
                                          1/sqrt(D) folded into Wq)
  Pm            = P * mask_tile          (VectorE 2x bf16)
  PmT           = transpose(Pm)          (16 PE transposes -> bf16 PSUM,
                                          2 [128,1024] copies to SBUF)
  o[128,129]    = sum_j PmT_j.T @ [v_j|1] (16 accumulating matmuls; col 128
                                          = softmax denominator)
  out           = o[:, :128] * 1/o[:,128] (VectorE reciprocal + ScalarE mul)
Output rows are written into an SG-tile group and stored every SG q-tiles.
The next batch's loads are issued at qt=T/2-2 and its setup (transposes +
k-projection) is emitted at qt=T/2+2 so the Tile scheduler interleaves it
into the current batch's pipeline instead of stalling PE at the boundary.
"""
import os
import sys

for _p in ("/opt/trn_rl_repo", "/root/.axon_site/_ro/trn_rl_repo"):
    if os.path.isdir(_p) and _p not in sys.path:
        sys.path.append(_p)

import numpy as np

import concourse.bass as bass
import concourse.tile as tile
from concourse import mybir
from concourse.masks import make_identity

B, N, D = 16, 2048, 128
NCORES = 8
NB = B // NCORES          # batches per core
T = N // 128              # 16 tiles of 128 rows
SCALE = float(1.0 / np.sqrt(D))
BF16 = mybir.dt.bfloat16
F32 = mybir.dt.float32

NACT = int(os.environ.get("KERNEL_NACT", "0"))      # PmT copy groups on ACT
LAG = int(os.environ.get("KERNEL_LAG", "1"))        # stage2 lag in q-tiles
MGROUP = int(os.environ.get("KERNEL_MGROUP", "1"))  # q-tiles per mask DMA
MASKB = int(os.environ.get("KERNEL_MASKB", "12"))
WORKB = int(os.environ.get("KERNEL_WORKB", "2"))
PREF = int(os.environ.get("KERNEL_PREF", "2"))      # mask groups issued ahead
POOLMUL = int(os.environ.get("KERNEL_POOLMUL", "0"))  # every n-th mul on Pool
SG = int(os.environ.get("KERNEL_SG", "4"))          # q-tiles per out store
PSB = int(os.environ.get("KERNEL_PSB", "2"))        # ps_pool bufs
# of every XBD q-tiles, this many use the xbar DMA transpose for PmT.
# MEASURED 2x WORSE at 1/2 (222us/rep): each dma_start_transpose occupies
# the Sync engine ~3.9us and serializes the pipeline. Keep 0 (all PE).
XBN = int(os.environ.get("KERNEL_XBN", "0"))
XBD = int(os.environ.get("KERNEL_XBD", "2"))
KPT = os.environ.get("KERNEL_KPT", "pt2")           # "pt2" | "pt" | "nat"
QPT = os.environ.get("KERNEL_QPT", "1") == "1"      # q "(p t)" layout
FAKEMASK = os.environ.get("KERNEL_FAKEMASK", "0") == "1"

MAXW = 1  # container walrus rejects >1 sync-wait per instruction


def _split_sync_waits(nc, maxw=MAXW):
    for bb in nc.main_func.blocks:
        out = []
        for ins in bb.instructions:
            si = ins.sync_info
            if si is not None and si.on_wait and len(si.on_wait) > maxw:
                waits = list(si.on_wait)
                extra, keep = waits[:-maxw], waits[-maxw:]
                while extra:
                    chunk, extra = extra[:maxw], extra[maxw:]
                    out.append(mybir.InstNoOp(
                        name=f"I-splitw-{nc.next_id()}",
                        engine=ins.engine, ins=[], outs=[],
                        text_hint="split_sync_waits", bass_nofuse=True,
                        sync_info=mybir.SyncInfo(on_wait=chunk, on_update=[]),
                    ))
                si.on_wait = keep
            out.append(ins)
        bb.instructions = out


def build_nc(reps: int = 1, split_waits: bool = True):
    nc = bass.Bass("TRN2", target_bir_lowering=False, debug=False,
                   num_devices=NCORES)
    qd = nc.dram_tensor("q", [NB, N, D], F32, kind="ExternalInput").ap()
    kd = nc.dram_tensor("k", [NB, N, D], F32, kind="ExternalInput").ap()
    vd = nc.dram_tensor("v", [NB, N, D], F32, kind="ExternalInput").ap()
    md = nc.dram_tensor("mask", [NB, N, N], F32, kind="ExternalInput").ap()
    wqd = nc.dram_tensor("Wq", [D, D], F32, kind="ExternalInput").ap()
    od = nc.dram_tensor("out", [NB, N, D], F32, kind="ExternalOutput").ap()

    NG = T // MGROUP  # mask groups per batch

    with tile.TileContext(nc) as tc:
        with (
            tc.tile_pool(name="const", bufs=1) as const,
            tc.tile_pool(name="stage", bufs=2) as stage,
            tc.tile_pool(name="perb", bufs=2) as perb,
            tc.tile_pool(name="maskp", bufs=MASKB) as maskp,
            tc.tile_pool(name="work", bufs=WORKB) as work,
            tc.tile_pool(name="outp", bufs=3) as outp,
            tc.tile_pool(name="ps_pool", bufs=PSB, space="PSUM") as ps_pool,
            tc.tile_pool(name="pt_pool", bufs=2, space="PSUM") as pt_pool,
            tc.tile_pool(name="po_pool", bufs=2, space="PSUM") as po_pool,
        ):
            ident = const.tile([128, 128], BF16, tag="ident")
            make_identity(nc, ident)
            wq_raw = const.tile([128, 128], BF16, tag="wq_raw")
            nc.gpsimd.dma_start(out=wq_raw, in_=wqd)  # fp32 -> bf16 cast
            wq_sc = const.tile([128, 128], BF16, tag="wq_sc")
            nc.vector.tensor_scalar_mul(wq_sc, wq_raw, SCALE)

            def transpose_blocks(dst3, src3, parity):
                # src3: [128, T, 128] staging; dst3: [128, T, 128] with
                # dst3[:, t, :] = src3[:, t, :].T
                G = 8
                for tg in range(T // G):
                    pt = pt_pool.tile([128, G * 128], BF16, tag="pt")
                    for j in range(G):
                        nc.tensor.transpose(
                            pt[:, j * 128:(j + 1) * 128],
                            src3[:, tg * G + j, :], ident)
                    dslice = dst3[:, tg * G:(tg + 1) * G, :]
                    if (tg + parity) % 2 == 0:
                        nc.scalar.copy(out=dslice, in_=pt)
                    else:
                        nc.vector.tensor_copy(out=dslice, in_=pt)

            def issue_loads(b):
                """Issue the q/k/v load DMAs for batch b (q via SWDGE bf16
                cast in the "(p t)" fat-descriptor layout; k/v via HWDGE
                fp32).  Called one batch ahead so the loads don't queue
                behind batch b's mask stream on the SWDGE queue."""
                if QPT:
                    qsrc = qd[b].rearrange("(p t) d -> p t d", t=T)
                else:
                    qsrc = qd[b].rearrange("(t p) d -> p t d", p=128)
                qstage = stage.tile([128, T, 128], BF16, tag="qstage")
                nc.gpsimd.dma_start(out=qstage, in_=qsrc)
                if KPT in ("pt", "pt2"):
                    ksrc = kd[b].rearrange("(p t) d -> p t d", t=T)
                    kstage = stage.tile([128, T, 128], BF16, tag="kstage")
                    nc.gpsimd.dma_start(out=kstage, in_=ksrc)
                    kstage_f = None
                else:
                    # HWDGE fp32 load + DVE cast (SWDGE 512B-desc gen too slow
                    # for the natural layout; strided PE rhs too slow for pt)
                    kstage_f = stage.tile([128, T, 128], F32, tag="kstage_f")
                    nc.sync.dma_start(
                        out=kstage_f,
                        in_=kd[b].rearrange("(t p) d -> p t d", p=128))
                    kstage = None
                # v also goes through the SWDGE fat-descriptor path: a 512B-
                # packet HWDGE load loses the SDMA round-robin ~8:1 against
                # the mask stream's fat packets and its tail crawls for ~25us,
                # stalling whichever engine queue holds the v1 build.  The
                # natural [m, d] tiles are recovered with two PE-transpose
                # rounds in emit_setup.
                vpt = stage.tile([128, T, 128], BF16, tag="vpt")
                nc.gpsimd.dma_start(
                    out=vpt, in_=vd[b].rearrange("(p t) d -> p t d", t=T))
                return qstage, kstage, kstage_f, vpt

            def transpose_blocks_ptfix(dst2, src3, parity=0):
                """Transpose "(p t)"-staged src into NATURAL column order:
                transpose block t yields columns {16p+t}; the PSUM->SBUF copy
                scatters them via a strided AP (inner stride 16 elems)."""
                G = 8
                dstv = dst2.rearrange("d (p s) -> d p s", p=128)
                for tg in range(T // G):
                    pt = pt_pool.tile([128, G * 128], BF16, tag="pt")
                    for j in range(G):
                        nc.tensor.transpose(
                            pt[:, j * 128:(j + 1) * 128],
                            src3[:, tg * G + j, :], ident)
                    src = pt.rearrange("d (j p) -> d p j", p=128)
                    dst = dstv[:, :, tg * G:(tg + 1) * G]
                    if (tg + parity) % 2 == 0:
                        nc.scalar.copy(out=dst, in_=src)
                    else:
                        nc.vector.tensor_copy(out=dst, in_=src)

            def emit_setup(b, loads):
                """Transpose q/k, project k, build v1 (+ ones column)."""
                qstage, kstage, kstage_f, vpt = loads
                if kstage is None:
                    kstage = stage.tile([128, T, 128], BF16, tag="kstage")
                    nc.vector.tensor_copy(out=kstage, in_=kstage_f)

                qT = perb.tile([128, N], BF16, tag="qT")
                if QPT:
                    transpose_blocks_ptfix(qT, qstage, parity=0)
                else:
                    transpose_blocks(
                        qT.rearrange("d (t p) -> d t p", p=128), qstage, 0)
                kT = stage.tile([128, N], BF16, tag="kT")
                if KPT == "pt2":
                    transpose_blocks_ptfix(kT, kstage, parity=1)
                else:
                    transpose_blocks(
                        kT.rearrange("d (t p) -> d t p", p=128), kstage, 1)

                # kpT[e, c] = sum_d (Wq[d, e]/sqrt(D)) * kT[d, c]
                kpT = perb.tile([128, N], BF16, tag="kpT")
                for c in range(N // 512):
                    pm = pt_pool.tile([128, 512], F32, tag="pt")
                    nc.tensor.matmul(pm, lhsT=wq_sc,
                                     rhs=kT[:, c * 512:(c + 1) * 512],
                                     start=True, stop=True)
                    if c % 2 == 0:
                        nc.scalar.copy(out=kpT[:, c * 512:(c + 1) * 512],
                                       in_=pm)
                    else:
                        nc.vector.tensor_copy(
                            out=kpT[:, c * 512:(c + 1) * 512], in_=pm)

                # v: "(p t)" staging -> natural vT -> per-j [m, d] tiles via a
                # second transpose round (ones col appended for the softmax
                # denominator).
                vT = stage.tile([128, N], BF16, tag="vT")
                transpose_blocks_ptfix(vT, vpt, parity=1)
                v1 = perb.tile([128, T, 129], BF16, tag="v1")
                G = 8
                for tg in range(T // G):
                    pt = pt_pool.tile([128, G * 128], BF16, tag="pt")
                    for j in range(G):
                        t_idx = tg * G + j
                        nc.tensor.transpose(
                            pt[:, j * 128:(j + 1) * 128],
                            vT[:, t_idx * 128:(t_idx + 1) * 128], ident)
                    dst = v1[:, tg * G:(tg + 1) * G, 0:128]
                    src = pt.rearrange("d (j p) -> d j p", p=128)
                    if tg % 2 == 0:
                        nc.vector.tensor_copy(out=dst, in_=src)
                    else:
                        nc.scalar.copy(out=dst, in_=src)
                nc.vector.memset(v1[:, :, 128:129], 1.0)
                return qT, kpT, v1

            for _rep in range(reps):
                mask_tiles = {}

                def issue_mask(gi):
                    # mask rows stay NATURAL (dense 2MB HBM blocks): the "(p
                    # s)" permuted layout reads a 16KB-every-128KB comb that
                    # drops HBM efficiency to ~76%.
                    b, g = gi // NG, gi % NG
                    msk = maskp.tile([128, MGROUP, N], BF16, tag="msk")
                    msrc = md[b].rearrange("(s p) c -> p s c", p=128)
                    nc.gpsimd.dma_start(
                        out=msk,
                        in_=msrc[:, g * MGROUP:(g + 1) * MGROUP, :])
                    mask_tiles[gi] = msk

                def emit_stage1(b, qt, qT, kpT, v1):
                    gi = b * NG + qt // MGROUP
                    if qt % MGROUP == 0:
                        ahead = 1 if gi == 0 else PREF
                        want = 0 if FAKEMASK else min(gi + ahead, NB * NG - 1)
                        for i in range(max(mask_tiles) + 1 if mask_tiles
                                       else 0, want + 1):
                            issue_mask(i)
                    msk = mask_tiles[0 if FAKEMASK else gi]

                    P = work.tile([128, N], BF16, tag="P")
                    Pm = work.tile([128, N], BF16, tag="Pm")
                    for h in range(2):
                        hsl = slice(h * 1024, (h + 1) * 1024)
                        ps = ps_pool.tile([128, 1024], F32, tag="ps")
                        for cc in range(2):
                            c = 2 * h + cc
                            rhs = kpT[:, c * 512:(c + 1) * 512]
                            nc.tensor.matmul(
                                ps[:, cc * 512:(cc + 1) * 512],
                                lhsT=qT[:, qt * 128:(qt + 1) * 128], rhs=rhs,
                                start=True, stop=True)
                        nc.scalar.activation(P[:, hsl], ps,
                                             mybir.ActivationFunctionType.Exp)
                        mi = qt * 2 + h
                        eng = (nc.gpsimd if POOLMUL and mi % POOLMUL ==
                               POOLMUL - 1 else nc.vector)
                        eng.tensor_mul(Pm[:, hsl], P[:, hsl],
                                       msk[:, qt % MGROUP, hsl])
                    return Pm

                def emit_stage2(b, qt, Pm, v1, ogroup):
                    PmT = work.tile([128, T, 128], BF16, tag="PmT")
                    if qt % XBD < XBN:
                        # xbar DMA transpose: SBUF->SBUF, no PSUM, no copy
                        nc.sync.dma_start_transpose(out=PmT, in_=Pm)
                    else:
                        G = 8
                        for tg in range(T // G):
                            pt = pt_pool.tile([128, G * 128], BF16, tag="pt")
                            for j in range(G):
                                t_idx = tg * G + j
                                nc.tensor.transpose(
                                    pt[:, j * 128:(j + 1) * 128],
                                    Pm[:, t_idx * 128:(t_idx + 1) * 128],
                                    ident)
                            dslice = PmT[:, tg * G:(tg + 1) * G, :]
                            if (tg + qt) % 4 < NACT:
                                nc.scalar.copy(out=dslice, in_=pt)
                            else:
                                nc.vector.tensor_copy(out=dslice, in_=pt)

                    po = po_pool.tile([128, 129], F32, tag="po")
                    for j in range(T):
                        nc.tensor.matmul(po, lhsT=PmT[:, j, :],
                                         rhs=v1[:, j, :],
                                         start=(j == 0), stop=(j == T - 1))

                    rinv = outp.tile([128, 1], F32, tag="rinv")
                    nc.vector.reciprocal(rinv, po[:, 128:129])
                    if qt % SG == 0:
                        ogroup[0] = outp.tile([128, SG, 128], F32,
                                              tag="osb", name="osb")
                    nc.scalar.mul(ogroup[0][:, qt % SG, :], po[:, 0:128], rinv)
                    if qt % SG == SG - 1:
                        qt0 = qt - SG + 1
                        odst = od[b].rearrange("(s p) d -> p s d", p=128)
                        nc.sync.dma_start(
                            out=odst[:, qt0:qt0 + SG, :], in_=ogroup[0])

                from collections import deque
                pending = deque()
                ogroup = [None]
                if FAKEMASK:
                    issue_mask(0)
                loads = issue_loads(0)
                built = emit_setup(0, loads)
                for b in range(NB):
                    qT, kpT, v1 = built
                    for qt in range(T):
                        Pm = emit_stage1(b, qt, qT, kpT, v1)
                        if qt == T // 2 - 2 and b + 1 < NB:
                            loads = issue_loads(b + 1)
                        # emit the next batch's setup build mid-batch so its
                        # PE transposes / kproj interleave into this batch's
                        # pipeline instead of stalling PE at the boundary
                        if qt == T // 2 + 2 and b + 1 < NB:
                            built = emit_setup(b + 1, loads)
                        pending.append((b, qt, Pm, v1))
                        if len(pending) > LAG:
                            emit_stage2(*pending.popleft(), ogroup)
                while pending:
                    emit_stage2(*pending.popleft(), ogroup)

    if split_waits:
        _split_sync_waits(nc)
    return nc


_CACHE = {}


def _get_nc(reps=1):
    if reps not in _CACHE:
        _CACHE[reps] = build_nc(reps)
    return _CACHE[reps]


def kernel(q, k, v, mask, Wq):
    from concourse.bass_utils import run_bass_kernel_spmd
    nc = _get_nc()
    in_maps = []
    for c in range(NCORES):
        sl = slice(c * NB, (c + 1) * NB)
        in_maps.append({
            "q": np.ascontiguousarray(q[sl]),
            "k": np.ascontiguousarray(k[sl]),
            "v": np.ascontiguousarray(v[sl]),
            "mask": np.ascontiguousarray(mask[sl]),
            "Wq": np.ascontiguousarray(Wq),
        })
    res = run_bass_kernel_spmd(nc, in_maps, list(range(NCORES)))
    out = np.concatenate([res.results[c]["out"] for c in range(NCORES)], axis=0)
    return out.astype(np.float32)
